# revision 25
# baseline (speedup 1.0000x reference)
"""Trainium2 Bass kernel for nn_NetworkBasic (2-layer SLAYER SNN).

Pipeline per layer (all per core, batch sharded 2/core across 8 cores):
  stage A (TensorE): temporal matmul  mid = data^T @ T   where
      T = c * P(srm-psp) @ D(2nd-difference), data is 0/1 in fp16,
      T supplied as fp16 hi+lo pair (2 accumulating matmuls).
      data chunks are transposed on TensorE ([128h,64t] -> [64t,128h]).
  stage B (TensorE): spatial 3x3 conv as 3 h-contraction matmuls
      (banded [128,128] H_dw matrices from the runtime conv weights)
      with w-shifted PSUM accumulation  ->  What ("w-hat") tensor.
  scan (VectorE): 2nd-order membrane recurrence, 2 ops/time-step:
      y_t     = (m[t] <= th) + 2d*m[t] + What[t+1]      (custom DVE op)
      m[t+1]  = -d^2 * m[t-1] + y_t                     (scalar_tensor_tensor)
  spikes (GpSimd): s = (m <= th)  bulk threshold.

Membrane math: the refractory alpha kernel ref[k] = A*k*d^k is realized as
an IIR via scaled variables (scale c = 1/(A*d) < 0, which flips >= to <=).
The FIR truncation tail of the reference is ~1e-4 and is ignored (validated:
~tens of spike flips out of 8.4M outputs).
"""

import os
import numpy as np

import concourse.bass as bass
import concourse.mybir as mybir
from concourse import bacc, bass_utils
from concourse.tile import TileContext
from concourse.masks import make_identity

F32 = mybir.dt.float32
F16 = mybir.dt.float16
AO = mybir.AluOpType

# ---------------- problem constants (hardcoded) ----------------
B_FULL, H, W, T = 16, 128, 64, 64
N_CORES = 8
B_LOC = B_FULL // N_CORES          # 2
BW = B_LOC * W                     # 128 (b,w) chunks per core
SP_FREE = BW * T                   # 8192 free elements ([128, 8192] tensors)

THETA = (30.0, 50.0)
TAU_SR = (1.0, 2.0)
TAU_REF = (1.0, 2.0)

SPATIAL_FP32R = os.environ.get("KERNEL_SPATIAL_FP32R", "0") == "1"
DEBUG_TAPS = os.environ.get("KERNEL_DEBUG_TAPS", "0") == "1"
ONE_LAYER = os.environ.get("KERNEL_ONE_LAYER", "0") == "1"
TRANSPOSE_MODE = os.environ.get("KERNEL_TRANSPOSE", "pe")  # "dma" | "pe"
NULL_KERNEL = os.environ.get("KERNEL_NULL", "0") == "1"


def _alpha_kernel(tau, mult, eps):
    vals = []
    for t in np.arange(0.0, float(T), 1.0):
        v = mult * t / tau * np.exp(1.0 - t / tau)
        if abs(v) < eps and t > tau:
            break
        vals.append(v)
    if len(vals) < 2:
        vals.append(0.0)
    return np.asarray(vals, np.float32)


SRM_K = [_alpha_kernel(TAU_SR[i], 1.0, 0.01) for i in range(2)]


def _layer_consts(layer):
    d = float(np.exp(-1.0 / TAU_REF[layer]))
    A = -2.0 * THETA[layer] * np.e / TAU_REF[layer]   # ref[k] = A*k*d^k
    c = 1.0 / (A * d)
    theta_hat = float(np.float32(c * THETA[layer]))
    return d, theta_hat


def _temporal_mat(layer):
    """[64,64] fp64 matrix:  what[t'] = sum_t data[t] * M[t, t']."""
    d, _ = _layer_consts(layer)
    A = -2.0 * THETA[layer] * np.e / TAU_REF[layer]
    c = 1.0 / (A * d)
    kern = SRM_K[layer].astype(np.float64)
    P = np.zeros((T, T))
    for t in range(T):
        for k in range(len(kern)):
            if t + k < T:
                P[t, t + k] = kern[k]
    D = np.zeros((T, T))
    for t in range(T):
        D[t, t] = 1.0
        if t + 1 < T:
            D[t, t + 1] = -2.0 * d
        if t + 2 < T:
            D[t, t + 2] = d * d
    return c * (P @ D)


def _hilo_f16(M):
    hi = M.astype(np.float16)
    lo = (M.astype(np.float32) - hi.astype(np.float32)).astype(np.float16)
    return hi, lo


def _h_mats(w):
    """w: [1,1,3,3] fp32 -> [3,128,128] fp32; Hm[dwi][h, hp] = w[h-hp+1, dwi]."""
    out = np.zeros((3, H, H), np.float32)
    for dwi in range(3):
        for dh in (-1, 0, 1):
            v = np.float32(w[0, 0, dh + 1, dwi])
            for hp in range(H):
                h = hp + dh
                if 0 <= h < H:
                    out[dwi, h, hp] = v
    return out


# ---------------- custom DVE op registration ----------------
_SNN_OP = None


def _register_snn_op():
    global _SNN_OP
    if _SNN_OP is not None:
        return _SNN_OP
    import concourse.dve_ops as dve_ops
    from concourse.dve_spec import Spec, Src0, Src1, C0, C1, lower
    from concourse.dve_uop import DveOpSpec

    name = "SNN_STEP_ANT"
    if name in dve_ops._SUB_OPCODE_FOR_NAME:
        _SNN_OP = next(op for op in dve_ops.OPS if op.name == name)
        return _SNN_OP

    # out = (s0 >= in0) + in0*s1 + in1
    body = (C0 >= Src0) + Src0 * C1 + Src1
    spec = Spec(
        body=body,
        reference=lambda in0, in1, s0, s1, imm2: (
            (np.float32(s0) >= in0).astype(np.float32)
            + in0 * np.float32(s1)
            + in1
        ).astype(np.float32),
    )
    row = 1 + len(dve_ops.OPS)
    shas = {}
    for ver in ("v3", "v4"):
        try:
            tmp = DveOpSpec(name=name, opcode=row, uops=lower(spec, ver=ver), rd1_en=True)
            shas[ver] = tmp.sha(ver)
        except Exception:
            pass
    op = dve_ops.DveOp(name, spec, subdim=False, uops_sha=shas)
    dve_ops.OPS.append(op)
    dve_ops._SUB_OPCODE_FOR_NAME[name] = row
    dve_ops.CUSTOM_DVE_SPECS[name] = spec
    _SNN_OP = op
    return op


# ---------------- bass kernel trace ----------------
def trace_kernel(nc, x_d, t_d, h_d, out_d):
    """x_d: [2,128,64,64] f32 dram; t_d: dict layer->(hi,lo) [64,64] f16 dram;
    h_d: dict layer->[3,128,128] f32 dram; out_d: [2,128,64,64] f32 dram."""
    snn_op = _register_snn_op()
    G = BW // 8          # 16 groups of 8 (b,w)-chunks
    NSLAB = T // 4       # 16 stage-B time slabs of 4

    with TileContext(nc) as tc:
        with (
            tc.tile_pool(name="const", bufs=1) as cpool,
            tc.tile_pool(name="big", bufs=1) as bpool,
            tc.tile_pool(name="xtg", bufs=3) as xtpool,
            tc.tile_pool(name="scan", bufs=2) as ypool,
            tc.tile_pool(name="ptrans", bufs=2, space="PSUM") as pt_pool,
            tc.tile_pool(name="pa", bufs=2, space="PSUM") as pa_pool,
            tc.tile_pool(name="pb", bufs=2, space="PSUM") as pb_pool,
        ):
            # constants (T matrices duplicated on both partition halves so
            # matmuls with lhsT at base-partition 64 have a matching rhs)
            if TRANSPOSE_MODE == "pe":
                ident = cpool.tile([H, H], F16)
                make_identity(nc, ident)
            tmats = {}
            for layer in (0, 1):
                thi = cpool.tile([2 * T, T], F16, tag=f"thi{layer}")
                tlo = cpool.tile([2 * T, T], F16, tag=f"tlo{layer}")
                for half in (0, 1):
                    nc.sync.dma_start(out=thi[half * T:(half + 1) * T, :],
                                      in_=t_d[layer][0].ap())
                    nc.sync.dma_start(out=tlo[half * T:(half + 1) * T, :],
                                      in_=t_d[layer][1].ap())
                tmats[layer] = (thi, tlo)
            hmats = {}
            for layer in (0, 1):
                hm = cpool.tile([H, 3 * H], F32, tag=f"h{layer}")
                nc.sync.dma_start(
                    out=hm[:, :].rearrange("p (k n) -> p k n", k=3),
                    in_=h_d[layer].ap().rearrange("k p n -> p k n"),
                )
                hmats[layer] = hm

            # input: x f32 [b,h,w,t] --sync-DMA--> staging f32 --DVE cast--> f16
            data0 = bpool.tile([H, SP_FREE], F16, tag="data")
            dview = data0[:, :].rearrange("p (b w t) -> p b w t", b=B_LOC, w=W)
            for b in range(B_LOC):
                for wh in range(4):
                    ws = slice(wh * 16, wh * 16 + 16)
                    stg = xtpool.tile([H, 16 * T], F32, tag="stg")
                    nc.sync.dma_start(out=stg, in_=x_d.ap()[b, :, ws, :])
                    nc.vector.tensor_copy(dview[:, b, ws, :], stg[:, :])

            if NULL_KERNEL:
                s2n = bpool.tile([H, SP_FREE], F32, tag="mid")
                nc.gpsimd.tensor_scalar(s2n, data0, 0.5, None, AO.is_le)
                s2nv = s2n[:, :].rearrange("p (b w t) -> p b w t", b=B_LOC, w=W)
                for b in range(B_LOC):
                    nc.sync.dma_start(out=out_d.ap()[b], in_=s2nv[:, b])
                return nc

            data = data0
            for layer in ((0,) if ONE_LAYER else (0, 1)):
                d, theta_hat = _layer_consts(layer)
                thi, tlo = tmats[layer]
                hm = hmats[layer]

                mid = bpool.tile([H, SP_FREE], F32, tag=f"mid")
                # ---- stage A: transposes + temporal matmuls ----
                scopeA = nc.enter_named_scope(f"stageA{layer}", False)
                for g in range(G):
                    pa = pa_pool.tile([H, 8 * T], F32, tag="pa")
                    if TRANSPOSE_MODE == "dma":
                        xts = []
                        for c2 in range(4):
                            pair = g * 4 + c2
                            xt = xtpool.tile([2 * T, H], F16, tag="xt")
                            eng = nc.sync
                            eng.dma_start_transpose(
                                xt, data[:, pair * 2 * T:(pair + 1) * 2 * T])
                            xts.append(xt)
                        def lhsT_of(c):
                            return xts[c // 2][(c % 2) * T:(c % 2 + 1) * T, :]
                    else:
                        ptr = pt_pool.tile([T, 8 * H], F16, tag="ptr")
                        ptr3 = ptr[:, :].rearrange("p (c n) -> p c n", c=8)
                        for c in range(8):
                            bw = g * 8 + c
                            nc.tensor.transpose(
                                ptr3[:, c, :], data[:, bw * T:(bw + 1) * T], ident)
                        xtg = xtpool.tile([T, 8 * H], F16, tag="xt")
                        nc.scalar.copy(xtg, ptr)
                        def lhsT_of(c):
                            return xtg[:, c * H:(c + 1) * H]
                    for c in range(8):
                        half = c % 2 if TRANSPOSE_MODE == "dma" else 0
                        lhsT = lhsT_of(c)
                        rhi = thi[half * T:(half + 1) * T, :]
                        rlo = tlo[half * T:(half + 1) * T, :]
                        nc.tensor.matmul(
                            pa[:, c * T:(c + 1) * T], lhsT, rhi,
                            start=True, stop=False, skip_group_check=True,
                        )
                        nc.tensor.matmul(
                            pa[:, c * T:(c + 1) * T], lhsT, rlo,
                            start=False, stop=True, skip_group_check=True,
                        )
                    nc.scalar.copy(mid[:, g * 512:(g + 1) * 512], pa)
                nc.leave_named_scope(f"stageA{layer}", scopeA[0], False)

                # ---- stage B: spatial conv, by time slab ----
                # What layout: [p, (slab16, b, w, t4)] -> contiguous evacs
                what = bpool.tile([H, SP_FREE], F32, tag="what")
                mview = mid[:, :].rearrange("p (b w t) -> p b w t", b=B_LOC, w=W)
                what5 = what[:, :].rearrange(
                    "p (s b w t) -> p s b w t", s=NSLAB, b=B_LOC, w=W)
                hm_mm, mv_mm = hm, mview
                for sp in range(NSLAB // 2):
                    pb = pb_pool.tile([H, 1024], F32, tag="pb")
                    pb5 = pb[:, :].rearrange(
                        "p (s b w t) -> p s b w t", s=2, b=B_LOC, w=W, t=4)
                    for ss in range(2):
                        s = sp * 2 + ss
                        ts = slice(s * 4, s * 4 + 4)
                        for b in range(B_LOC):
                            nc.tensor.matmul(
                                pb5[:, ss, b, :, :], hm_mm[:, H:2 * H],
                                mv_mm[:, b, :, ts],
                                start=True, stop=False, skip_group_check=True,
                            )
                            nc.tensor.matmul(
                                pb5[:, ss, b, 1:, :], hm_mm[:, 0:H],
                                mv_mm[:, b, :-1, ts],
                                start=False, stop=False, skip_group_check=True,
                            )
                            nc.tensor.matmul(
                                pb5[:, ss, b, :-1, :], hm_mm[:, 2 * H:3 * H],
                                mv_mm[:, b, 1:, ts],
                                start=False, stop=True, skip_group_check=True,
                            )
                    nc.scalar.copy(
                        what[:, sp * 1024:(sp + 1) * 1024], pb)

                # ---- scan ----
                scopeS = nc.enter_named_scope(f"scan{layer}", False)
                mh = bpool.tile([H, SP_FREE], F32, tag=f"mh{layer}")
                mh3 = mh[:, :].rearrange("p (bw t) -> p bw t", t=T)
                wS = what[:, :].rearrange("p (s bw t) -> p s bw t", s=NSLAB, t=4)

                def wslice(t):
                    return wS[:, t // 4, :, t % 4]

                nc.scalar.copy(mh3[:, :, 0], wslice(0))
                two_d = float(np.float32(2.0 * d))
                md2 = float(np.float32(-(d * d)))
                for t in range(T - 1):
                    if t == 0:
                        nc.vector._custom_dve(
                            snn_op, out=mh3[:, :, 1], in0=mh3[:, :, 0],
                            in1=wslice(1), s0=theta_hat, s1=two_d,
                        )
                    else:
                        y = ypool.tile([H, BW], F32, tag="y")
                        nc.vector._custom_dve(
                            snn_op, out=y, in0=mh3[:, :, t],
                            in1=wslice(t + 1), s0=theta_hat, s1=two_d,
                        )
                        nc.vector.scalar_tensor_tensor(
                            mh3[:, :, t + 1], mh3[:, :, t - 1], md2, y,
                            AO.mult, AO.add,
                        )

                nc.leave_named_scope(f"scan{layer}", scopeS[0], False)
                # ---- debug taps ----
                if DEBUG_TAPS and layer == 0:
                    for nm, tile_ in (("mid1", mid), ("what1", what), ("mh1", mh)):
                        dbg = nc.dram_tensor(nm, [H, SP_FREE], F32, kind="ExternalOutput")
                        nc.sync.dma_start(out=dbg.ap(), in_=tile_[:, :])

                # ---- spikes ----
                if layer == 0:
                    s1 = bpool.tile([H, SP_FREE], F16, tag="data")
                    nc.vector.tensor_scalar(
                        s1, mh, theta_hat, None, AO.is_le,
                    )
                    data = s1
                else:
                    s2 = bpool.tile([H, SP_FREE], F32, tag="mid")
                    nc.vector.tensor_scalar(
                        s2, mh, theta_hat, None, AO.is_le,
                    )
                    s2v = s2[:, :].rearrange("p (b w t) -> p b w t", b=B_LOC, w=W)
                    for b in range(B_LOC):
                        nc.sync.dma_start(out=out_d.ap()[b], in_=s2v[:, b])
    return nc


_BUILT = {}


def _build():
    global _BUILT
    key = (NULL_KERNEL, TRANSPOSE_MODE, ONE_LAYER, DEBUG_TAPS)
    if key in _BUILT:
        return _BUILT[key]
    nc = bacc.Bacc("TRN2", debug=False)
    x_d = nc.dram_tensor("x", [B_LOC, H, W, T], F32, kind="ExternalInput")
    t_d, h_d = {}, {}
    for layer in (0, 1):
        t_d[layer] = (
            nc.dram_tensor(f"t{layer}hi", [T, T], F16, kind="ExternalInput"),
            nc.dram_tensor(f"t{layer}lo", [T, T], F16, kind="ExternalInput"),
        )
        h_d[layer] = nc.dram_tensor(f"h{layer}", [3, H, H], F32, kind="ExternalInput")
    out_d = nc.dram_tensor("out", [B_LOC, H, W, T], F32, kind="ExternalOutput")
    trace_kernel(nc, x_d, t_d, h_d, out_d)
    nc.compile()
    _BUILT[key] = nc
    return nc


def _host_inputs(conv1_w, conv2_w):
    """Common (replicated) input tensors, computed on host."""
    ins = {}
    for layer, w in ((0, conv1_w), (1, conv2_w)):
        hi, lo = _hilo_f16(_temporal_mat(layer))
        ins[f"t{layer}hi"] = hi
        ins[f"t{layer}lo"] = lo
        ins[f"h{layer}"] = _h_mats(np.asarray(w, np.float32))
    return ins


def kernel(spikeInput, conv1_w, conv2_w):
    x = np.ascontiguousarray(np.asarray(spikeInput, np.float32).reshape(B_FULL, H, W, T))
    common = _host_inputs(conv1_w, conv2_w)
    nc = _build()
    in_maps = []
    for c in range(N_CORES):
        m = dict(common)
        m["x"] = np.ascontiguousarray(x[c * B_LOC:(c + 1) * B_LOC])
        in_maps.append(m)
    res = bass_utils.run_bass_kernel_spmd(nc, in_maps, core_ids=list(range(N_CORES)))
    out = np.concatenate([r["out"] for r in res.results], axis=0)
    return out.astype(np.float32)


# revision 27
# speedup vs baseline: 1.3547x; 1.3547x over previous
"""Trainium2 Bass kernel for nn_NetworkBasic (2-layer SLAYER SNN).

Pipeline per layer (all per core, batch sharded 2/core across 8 cores):
  stage A (TensorE): temporal matmul  mid = data^T @ T   where
      T = c * P(srm-psp) @ D(2nd-difference), data is 0/1 in fp16,
      T supplied as fp16 hi+lo pair (2 accumulating matmuls).
      data chunks are transposed on TensorE ([128h,64t] -> [64t,128h]).
  stage B (TensorE): spatial 3x3 conv as 3 h-contraction matmuls
      (banded [128,128] H_dw matrices from the runtime conv weights)
      with w-shifted PSUM accumulation  ->  What ("w-hat") tensor.
  scan (VectorE): 2nd-order membrane recurrence, 2 ops/time-step:
      y_t     = (m[t] <= th) + 2d*m[t] + What[t+1]      (custom DVE op)
      m[t+1]  = -d^2 * m[t-1] + y_t                     (scalar_tensor_tensor)
  spikes (GpSimd): s = (m <= th)  bulk threshold.

Membrane math: the refractory alpha kernel ref[k] = A*k*d^k is realized as
an IIR via scaled variables (scale c = 1/(A*d) < 0, which flips >= to <=).
The FIR truncation tail of the reference is ~1e-4 and is ignored (validated:
~tens of spike flips out of 8.4M outputs).
"""

import os
import numpy as np

import concourse.bass as bass
import concourse.mybir as mybir
from concourse import bacc, bass_utils
from concourse.tile import TileContext
from concourse.masks import make_identity

F32 = mybir.dt.float32
F16 = mybir.dt.float16
AO = mybir.AluOpType

# ---------------- problem constants (hardcoded) ----------------
B_FULL, H, W, T = 16, 128, 64, 64
N_CORES = 8
B_LOC = B_FULL // N_CORES          # 2
BW = B_LOC * W                     # 128 (b,w) chunks per core
SP_FREE = BW * T                   # 8192 free elements ([128, 8192] tensors)

THETA = (30.0, 50.0)
TAU_SR = (1.0, 2.0)
TAU_REF = (1.0, 2.0)

SPATIAL_FP32R = os.environ.get("KERNEL_SPATIAL_FP32R", "0") == "1"
DEBUG_TAPS = os.environ.get("KERNEL_DEBUG_TAPS", "0") == "1"
ONE_LAYER = os.environ.get("KERNEL_ONE_LAYER", "0") == "1"
TRANSPOSE_MODE = os.environ.get("KERNEL_TRANSPOSE", "pe")  # "dma" | "pe"
NULL_KERNEL = os.environ.get("KERNEL_NULL", "0") == "1"


def _alpha_kernel(tau, mult, eps):
    vals = []
    for t in np.arange(0.0, float(T), 1.0):
        v = mult * t / tau * np.exp(1.0 - t / tau)
        if abs(v) < eps and t > tau:
            break
        vals.append(v)
    if len(vals) < 2:
        vals.append(0.0)
    return np.asarray(vals, np.float32)


SRM_K = [_alpha_kernel(TAU_SR[i], 1.0, 0.01) for i in range(2)]


def _layer_consts(layer):
    d = float(np.exp(-1.0 / TAU_REF[layer]))
    A = -2.0 * THETA[layer] * np.e / TAU_REF[layer]   # ref[k] = A*k*d^k
    c = 1.0 / (A * d)
    theta_hat = float(np.float32(c * THETA[layer]))
    return d, theta_hat


def _temporal_mat(layer):
    """[64,64] fp64 matrix:  what[t'] = sum_t data[t] * M[t, t']."""
    d, _ = _layer_consts(layer)
    A = -2.0 * THETA[layer] * np.e / TAU_REF[layer]
    c = 1.0 / (A * d)
    kern = SRM_K[layer].astype(np.float64)
    P = np.zeros((T, T))
    for t in range(T):
        for k in range(len(kern)):
            if t + k < T:
                P[t, t + k] = kern[k]
    D = np.zeros((T, T))
    for t in range(T):
        D[t, t] = 1.0
        if t + 1 < T:
            D[t, t + 1] = -2.0 * d
        if t + 2 < T:
            D[t, t + 2] = d * d
    return c * (P @ D)


def _hilo_f16(M):
    hi = M.astype(np.float16)
    lo = (M.astype(np.float32) - hi.astype(np.float32)).astype(np.float16)
    return hi, lo


def _hilo_f16_blockdiag(M):
    hi, lo = _hilo_f16(M)
    bhi = np.zeros((2 * T, 2 * T), np.float16)
    blo = np.zeros((2 * T, 2 * T), np.float16)
    for i in (0, 1):
        bhi[i * T:(i + 1) * T, i * T:(i + 1) * T] = hi
        blo[i * T:(i + 1) * T, i * T:(i + 1) * T] = lo
    return bhi, blo


def _h_mats(w):
    """w: [1,1,3,3] fp32 -> [3,128,128] fp32; Hm[dwi][h, hp] = w[h-hp+1, dwi]."""
    out = np.zeros((3, H, H), np.float32)
    for dwi in range(3):
        for dh in (-1, 0, 1):
            v = np.float32(w[0, 0, dh + 1, dwi])
            for hp in range(H):
                h = hp + dh
                if 0 <= h < H:
                    out[dwi, h, hp] = v
    return out


# ---------------- custom DVE op registration ----------------
_SNN_OP = None


def _register_snn_op():
    global _SNN_OP
    if _SNN_OP is not None:
        return _SNN_OP
    import concourse.dve_ops as dve_ops
    from concourse.dve_spec import Spec, Src0, Src1, C0, C1, lower
    from concourse.dve_uop import DveOpSpec

    name = "SNN_STEP_ANT"
    if name in dve_ops._SUB_OPCODE_FOR_NAME:
        _SNN_OP = next(op for op in dve_ops.OPS if op.name == name)
        return _SNN_OP

    # out = (s0 >= in0) + in0*s1 + in1
    body = (C0 >= Src0) + Src0 * C1 + Src1
    spec = Spec(
        body=body,
        reference=lambda in0, in1, s0, s1, imm2: (
            (np.float32(s0) >= in0).astype(np.float32)
            + in0 * np.float32(s1)
            + in1
        ).astype(np.float32),
    )
    row = 1 + len(dve_ops.OPS)
    shas = {}
    for ver in ("v3", "v4"):
        try:
            tmp = DveOpSpec(name=name, opcode=row, uops=lower(spec, ver=ver), rd1_en=True)
            shas[ver] = tmp.sha(ver)
        except Exception:
            pass
    op = dve_ops.DveOp(name, spec, subdim=False, uops_sha=shas)
    dve_ops.OPS.append(op)
    dve_ops._SUB_OPCODE_FOR_NAME[name] = row
    dve_ops.CUSTOM_DVE_SPECS[name] = spec
    _SNN_OP = op
    return op


# ---------------- bass kernel trace ----------------
def trace_kernel(nc, x_d, t_d, h_d, out_d):
    """x_d: [2,128,64,64] f32 dram; t_d: dict layer->(hi,lo) [64,64] f16 dram;
    h_d: dict layer->[3,128,128] f32 dram; out_d: [2,128,64,64] f32 dram."""
    snn_op = _register_snn_op()
    G = BW // 8          # 16 groups of 8 (b,w)-chunks
    NSLAB = T // 4       # 16 stage-B time slabs of 4

    with TileContext(nc) as tc:
        with (
            tc.tile_pool(name="const", bufs=1) as cpool,
            tc.tile_pool(name="big", bufs=1) as bpool,
            tc.tile_pool(name="xtg", bufs=3) as xtpool,
            tc.tile_pool(name="scan", bufs=2) as ypool,
            tc.tile_pool(name="ptrans", bufs=2, space="PSUM") as pt_pool,
            tc.tile_pool(name="pa", bufs=2, space="PSUM") as pa_pool,
            tc.tile_pool(name="pb", bufs=2, space="PSUM") as pb_pool,
        ):
            # constants (T matrices duplicated on both partition halves so
            # matmuls with lhsT at base-partition 64 have a matching rhs)
            ident = cpool.tile([H, H], F16)
            make_identity(nc, ident)
            tmats = {}
            for layer in (0, 1):
                thi = cpool.tile([2 * T, 2 * T], F16, tag=f"thi{layer}")
                tlo = cpool.tile([2 * T, 2 * T], F16, tag=f"tlo{layer}")
                nc.sync.dma_start(out=thi, in_=t_d[layer][0].ap())
                nc.sync.dma_start(out=tlo, in_=t_d[layer][1].ap())
                tmats[layer] = (thi, tlo)
            hmats = {}
            for layer in (0, 1):
                hm = cpool.tile([H, 3 * H], F32, tag=f"h{layer}")
                nc.sync.dma_start(
                    out=hm[:, :].rearrange("p (k n) -> p k n", k=3),
                    in_=h_d[layer].ap().rearrange("k p n -> p k n"),
                )
                hmats[layer] = hm

            # input: x f32 [b,h,w,t] --sync-DMA--> staging f32 --DVE cast--> f16
            data0 = bpool.tile([H, SP_FREE], F16, tag="data")
            dview = data0[:, :].rearrange("p (b w t) -> p b w t", b=B_LOC, w=W)
            for b in range(B_LOC):
                for wh in range(4):
                    ws = slice(wh * 16, wh * 16 + 16)
                    stg = xtpool.tile([H, 16 * T], F32, tag="stg")
                    nc.sync.dma_start(out=stg, in_=x_d.ap()[b, :, ws, :])
                    nc.vector.tensor_copy(dview[:, b, ws, :], stg[:, :])

            if NULL_KERNEL:
                s2n = bpool.tile([H, SP_FREE], F32, tag="mid")
                nc.gpsimd.tensor_scalar(s2n, data0, 0.5, None, AO.is_le)
                s2nv = s2n[:, :].rearrange("p (b w t) -> p b w t", b=B_LOC, w=W)
                for b in range(B_LOC):
                    nc.sync.dma_start(out=out_d.ap()[b], in_=s2nv[:, b])
                return nc

            data = data0
            for layer in ((0,) if ONE_LAYER else (0, 1)):
                d, theta_hat = _layer_consts(layer)
                thi, tlo = tmats[layer]
                hm = hmats[layer]

                mid = bpool.tile([H, SP_FREE], F32, tag=f"mid")
                # ---- stage A: pair transposes + block-diag temporal matmuls ----
                scopeA = nc.enter_named_scope(f"stageA{layer}", False)
                for g in range(G):
                    pa = pa_pool.tile([H, 8 * T], F32, tag="pa")
                    ptr = pt_pool.tile([H, 4 * H], F16, tag="ptr")
                    for c2 in range(4):
                        pair = g * 4 + c2
                        nc.tensor.transpose(
                            ptr[:, c2 * H:(c2 + 1) * H],
                            data[:, pair * 2 * T:(pair + 1) * 2 * T],
                            ident,
                        )
                    xtg = xtpool.tile([H, 4 * H], F16, tag="xt")
                    nc.scalar.copy(xtg, ptr)
                    for c2 in range(4):
                        lhsT = xtg[:, c2 * H:(c2 + 1) * H]
                        nc.tensor.matmul(
                            pa[:, c2 * H:(c2 + 1) * H], lhsT, thi,
                            start=True, stop=False, skip_group_check=True,
                        )
                        nc.tensor.matmul(
                            pa[:, c2 * H:(c2 + 1) * H], lhsT, tlo,
                            start=False, stop=True, skip_group_check=True,
                        )
                    nc.scalar.copy(mid[:, g * 512:(g + 1) * 512], pa)
                nc.leave_named_scope(f"stageA{layer}", scopeA[0], False)

                # ---- stage B: spatial conv, by time slab ----
                # What layout: [p, (slab16, b, w, t4)] -> contiguous evacs
                what = bpool.tile([H, SP_FREE], F32, tag="what")
                mview = mid[:, :].rearrange("p (b w t) -> p b w t", b=B_LOC, w=W)
                what5 = what[:, :].rearrange(
                    "p (s b w t) -> p s b w t", s=NSLAB, b=B_LOC, w=W)
                hm_mm, mv_mm = hm, mview
                for sp in range(NSLAB // 2):
                    pb = pb_pool.tile([H, 1024], F32, tag="pb")
                    pb5 = pb[:, :].rearrange(
                        "p (s b w t) -> p s b w t", s=2, b=B_LOC, w=W, t=4)
                    for ss in range(2):
                        s = sp * 2 + ss
                        ts = slice(s * 4, s * 4 + 4)
                        for b in range(B_LOC):
                            nc.tensor.matmul(
                                pb5[:, ss, b, :, :], hm_mm[:, H:2 * H],
                                mv_mm[:, b, :, ts],
                                start=True, stop=False, skip_group_check=True,
                            )
                            nc.tensor.matmul(
                                pb5[:, ss, b, 1:, :], hm_mm[:, 0:H],
                                mv_mm[:, b, :-1, ts],
                                start=False, stop=False, skip_group_check=True,
                            )
                            nc.tensor.matmul(
                                pb5[:, ss, b, :-1, :], hm_mm[:, 2 * H:3 * H],
                                mv_mm[:, b, 1:, ts],
                                start=False, stop=True, skip_group_check=True,
                            )
                    nc.scalar.copy(
                        what[:, sp * 1024:(sp + 1) * 1024], pb)

                # ---- scan ----
                scopeS = nc.enter_named_scope(f"scan{layer}", False)
                mh = bpool.tile([H, SP_FREE], F32, tag=f"mh{layer}")
                mh3 = mh[:, :].rearrange("p (bw t) -> p bw t", t=T)
                wS = what[:, :].rearrange("p (s bw t) -> p s bw t", s=NSLAB, t=4)

                def wslice(t):
                    return wS[:, t // 4, :, t % 4]

                nc.scalar.copy(mh3[:, :, 0], wslice(0))
                two_d = float(np.float32(2.0 * d))
                md2 = float(np.float32(-(d * d)))
                for t in range(T - 1):
                    if t == 0:
                        nc.vector._custom_dve(
                            snn_op, out=mh3[:, :, 1], in0=mh3[:, :, 0],
                            in1=wslice(1), s0=theta_hat, s1=two_d,
                        )
                    else:
                        y = ypool.tile([H, BW], F32, tag="y")
                        nc.vector._custom_dve(
                            snn_op, out=y, in0=mh3[:, :, t],
                            in1=wslice(t + 1), s0=theta_hat, s1=two_d,
                        )
                        nc.vector.scalar_tensor_tensor(
                            mh3[:, :, t + 1], mh3[:, :, t - 1], md2, y,
                            AO.mult, AO.add,
                        )

                nc.leave_named_scope(f"scan{layer}", scopeS[0], False)
                # ---- debug taps ----
                if DEBUG_TAPS and layer == 0:
                    for nm, tile_ in (("mid1", mid), ("what1", what), ("mh1", mh)):
                        dbg = nc.dram_tensor(nm, [H, SP_FREE], F32, kind="ExternalOutput")
                        nc.sync.dma_start(out=dbg.ap(), in_=tile_[:, :])

                # ---- spikes ----
                if layer == 0:
                    s1 = bpool.tile([H, SP_FREE], F16, tag="data")
                    nc.vector.tensor_scalar(
                        s1, mh, theta_hat, None, AO.is_le,
                    )
                    data = s1
                else:
                    s2 = bpool.tile([H, SP_FREE], F32, tag="mid")
                    nc.vector.tensor_scalar(
                        s2, mh, theta_hat, None, AO.is_le,
                    )
                    s2v = s2[:, :].rearrange("p (b w t) -> p b w t", b=B_LOC, w=W)
                    for b in range(B_LOC):
                        nc.sync.dma_start(out=out_d.ap()[b], in_=s2v[:, b])
    return nc


_BUILT = {}


def _build():
    global _BUILT
    key = (NULL_KERNEL, TRANSPOSE_MODE, ONE_LAYER, DEBUG_TAPS)
    if key in _BUILT:
        return _BUILT[key]
    nc = bacc.Bacc("TRN2", debug=False)
    x_d = nc.dram_tensor("x", [B_LOC, H, W, T], F32, kind="ExternalInput")
    t_d, h_d = {}, {}
    for layer in (0, 1):
        t_d[layer] = (
            nc.dram_tensor(f"t{layer}hi", [2 * T, 2 * T], F16, kind="ExternalInput"),
            nc.dram_tensor(f"t{layer}lo", [2 * T, 2 * T], F16, kind="ExternalInput"),
        )
        h_d[layer] = nc.dram_tensor(f"h{layer}", [3, H, H], F32, kind="ExternalInput")
    out_d = nc.dram_tensor("out", [B_LOC, H, W, T], F32, kind="ExternalOutput")
    trace_kernel(nc, x_d, t_d, h_d, out_d)
    nc.compile()
    _BUILT[key] = nc
    return nc


def _host_inputs(conv1_w, conv2_w):
    """Common (replicated) input tensors, computed on host."""
    ins = {}
    for layer, w in ((0, conv1_w), (1, conv2_w)):
        hi, lo = _hilo_f16_blockdiag(_temporal_mat(layer))
        ins[f"t{layer}hi"] = hi
        ins[f"t{layer}lo"] = lo
        ins[f"h{layer}"] = _h_mats(np.asarray(w, np.float32))
    return ins


def kernel(spikeInput, conv1_w, conv2_w):
    x = np.ascontiguousarray(np.asarray(spikeInput, np.float32).reshape(B_FULL, H, W, T))
    common = _host_inputs(conv1_w, conv2_w)
    nc = _build()
    in_maps = []
    for c in range(N_CORES):
        m = dict(common)
        m["x"] = np.ascontiguousarray(x[c * B_LOC:(c + 1) * B_LOC])
        in_maps.append(m)
    res = bass_utils.run_bass_kernel_spmd(nc, in_maps, core_ids=list(range(N_CORES)))
    out = np.concatenate([r["out"] for r in res.results], axis=0)
    return out.astype(np.float32)
